# revision 73
# baseline (speedup 1.0000x reference)
"""MultiHeadAttention forward on 8 Trainium2 NeuronCores.

Tensor-parallel over heads: each core owns 2 of 16 heads (d_loc=256 of the
2048 QKV output columns, and the matching 256 rows of Wo). Each core
computes a full-shape partial output (bf16); the host sums the 8 partials
and adds bo + bv@Wo.

Problem shape: x [2, 2048, 2048], 16 heads, d_k = 128, fp32; all on-chip
compute in bf16 (scores/AV accumulate in fp32 PSUM).

Schedule notes (all engines are in-order; the point is a dense PE stream):
  - DMA: per-dma_start cost (~0.6 us issue + transfers that serialize per
    HWDGE ring) dominates over descriptor count, so every load/store is
    one full-128-partition dma_start. x rides the Sync ring, weights the
    Scalar ring, so the two startup streams issue in parallel. All of
    batch-0's projection chunks are emitted tile-interleaved (j0, j2,
    v-t0..t3 per x tile, 6 concurrent PSUM chains) so PE consumption pace
    matches DMA arrival pace, and a short ones-matmul warm-up ramps the
    PE p-state while the first w/x transfers are in flight.
  - Scores are computed transposed ST[tk, tq] in [128,1024] PSUM pairs;
    each ScalarE exp covers 1024 columns (fp32 PSUM in, bf16 out).
  - Softmax denominator: ALL exp folds on DVE (bf16 2x, ~0.43 us each) —
    concurrent Pool tensor ops degrade DVE ~3x via an SBUF port conflict,
    so Pool is kept quiet; two accumulating ones-matmuls reduce over
    partitions. Each unit's finalize is deferred into the next unit (p2).
  - QKV bias-adds and V-projection PSUM->SBUF copies run as Identity
    activations on the Scalar engine (idle during projection phases);
    exp and Identity share one ACT table so there are no table reloads.
  - O-projection quanta interleave into attention units; their q==1
    PSUM->SBUF casts go to Scalar, q==0 to DVE. The tail (last chunk)
    additionally alternates store rings (sync/scalar) and PSUM pools
    (ps_o/ps_av) to drain ~2x faster after the final finalize.
"""

import functools
from contextlib import ExitStack

import numpy as np

D_MODEL = 2048
NUM_HEADS = 16
DK = 128
B = 2
T = 2048
BT = B * T
N_CORES = 8
H_LOC = NUM_HEADS // N_CORES  # 2 heads per core
D_LOC = H_LOC * DK  # 256
C_TILES = D_MODEL // 128  # 16
TQ = 512  # tq chunk width
NCH = T // TQ  # 4 chunks per batch
TK_TILES = T // 128  # 16
NPAIR = TK_TILES // 2  # 8 score-pairs per attention unit


class OEmitter:
    """Output projection for one (batch, chunk): 16 quanta of 2 matmuls +
    1 DVE copy, one row-split ystage DMA per (t, half)."""

    def __init__(self, ctx, b, ch, avT, row_split=1, cast_engs=("vector", "vector")):
        self.ctx = ctx
        self.b = b
        self.avT = avT  # snapshot: O of batch b reads batch b's avT tiles
        self.items = [
            (t, half, q)
            for t in range(ch * 4, (ch + 1) * 4)
            for half in range(2)
            for q in range(2)
        ]
        self.idx = 0
        self.row_split = row_split
        # Which engine runs the PSUM->SBUF cast for q==0 / q==1 quanta:
        # "gpsimd" offloads to the otherwise-idle Pool engine inside
        # attention windows; "scalar" is for the tail (exps done by then).
        self.cast_engs = cast_engs
        self.store_alt = False
        # Tail-only: q==1 quanta accumulate in the (idle by then) ps_av
        # pool so the last chunk is not serialized on ps_o's two buffers.
        self.psum_alt = False
        self.ys = None

    def emit(self, k):
        c = self.ctx
        nc = c["nc"]
        for _ in range(k):
            if self.idx >= len(self.items):
                return
            t, half, q = self.items[self.idx]
            self.idx += 1
            if q == 0:
                self.ys = c["y_pool"].tile(
                    [128, 1024], c["bf16"], tag="ys", name=f"ys{self.b}_{t}_{half}"
                )
            pool = c["ps_av"] if (self.psum_alt and q == 1) else c["ps_o"]
            ps = pool.tile(
                [128, TQ],
                c["f32"],
                tag="av" if (self.psum_alt and q == 1) else "o",
                name=f"pso{self.b}_{t}_{half}_{q}",
            )
            for d in range(2):
                nc.tensor.matmul(
                    ps,
                    self.avT[d][:, t * 128 : (t + 1) * 128],
                    c["wo_tiles"][d][:, (half * 2 + q) * TQ : (half * 2 + q + 1) * TQ],
                    start=(d == 0),
                    stop=(d == 1),
                )
            eng = self.cast_engs[q]
            if self.psum_alt and self.idx <= 6:
                # Tail head: Scalar is still draining the last unit's exps,
                # so the first few casts all go to DVE (free once the last
                # fold chain retires).
                eng = "vector"
            if eng == "scalar":
                from concourse import mybir as _mybir

                nc.scalar.activation(
                    self.ys[:, q * TQ : (q + 1) * TQ],
                    ps,
                    _mybir.ActivationFunctionType.Identity,
                )
            elif eng == "gpsimd":
                nc.gpsimd.tensor_copy(self.ys[:, q * TQ : (q + 1) * TQ], ps)
            else:
                nc.vector.tensor_copy(self.ys[:, q * TQ : (q + 1) * TQ], ps)
            if q == 1:
                row0 = self.b * T + t * 128
                rs = 128 // self.row_split
                ring = nc.scalar if (self.store_alt and half == 1) else nc.sync
                for s in range(self.row_split):
                    ring.dma_start(
                        out=c["y"][
                            row0 + s * rs : row0 + (s + 1) * rs,
                            half * 1024 : (half + 1) * 1024,
                        ],
                        in_=self.ys[s * rs : (s + 1) * rs, :],
                    )

    def remaining(self):
        return len(self.items) - self.idx


def _body(ctx_stack, tc, xT, wqkv, bqk, wo, y):
    import concourse.bass as bass  # noqa: F401
    from concourse import mybir

    nc = tc.nc
    f32 = mybir.dt.float32
    bf16 = mybir.dt.bfloat16
    Exp = mybir.ActivationFunctionType.Exp
    inv_sqrt_dk = 1.0 / float(np.sqrt(DK))

    # ---------------- pools ----------------
    wpool = ctx_stack.enter_context(tc.tile_pool(name="wpool", bufs=1))
    xw_pool = ctx_stack.enter_context(tc.tile_pool(name="xw_pool", bufs=1))
    qkv_pool = ctx_stack.enter_context(tc.tile_pool(name="qkv_pool", bufs=1))
    av_pool = ctx_stack.enter_context(tc.tile_pool(name="av_pool", bufs=1))
    es_pool = ctx_stack.enter_context(tc.tile_pool(name="es_pool", bufs=6))
    acc_pool = ctx_stack.enter_context(tc.tile_pool(name="acc_pool", bufs=2))
    rc_pool = ctx_stack.enter_context(tc.tile_pool(name="rc_pool", bufs=2))
    y_pool = ctx_stack.enter_context(tc.tile_pool(name="y_pool", bufs=4))

    ps_wide = ctx_stack.enter_context(
        tc.tile_pool(name="ps_wide", bufs=2, space="PSUM")
    )
    ps_av = ctx_stack.enter_context(tc.tile_pool(name="ps_av", bufs=2, space="PSUM"))
    ps_o = ctx_stack.enter_context(tc.tile_pool(name="ps_o", bufs=2, space="PSUM"))

    # ---------------- resident tensors ----------------
    # Issue cost dominates the startup (~0.65 us per dma_start per HWDGE
    # ring): one full-128-partition dma_start per tile, x chunk-0 on the
    # Sync ring and w on the Scalar ring so the two streams issue in
    # parallel.
    w_tiles = []
    xt_b0 = []
    for i in range(C_TILES):
        xt0 = xw_pool.tile([128, T], bf16, tag=f"xw{i}", name=f"xw0_{i}")
        nc.sync.dma_start(
            out=xt0[:, 0:TQ], in_=xT[i * 128 : (i + 1) * 128, 0:TQ]
        )
        xt_b0.append(xt0)
        wt = wpool.tile([128, 3 * D_LOC], bf16, tag=f"w{i}", name=f"w{i}")
        nc.scalar.dma_start(out=wt, in_=wqkv[i * 128 : (i + 1) * 128, :])
        w_tiles.append(wt)
    # rest of batch-0 x behind chunk 0, chunk-1 pieces first so QKT(ch1)
    # is not gated by the full-wave transfer (transfers serialize per
    # HWDGE ring; all x stays on sync, weights on scalar).
    for lo, hi in ((TQ, 2 * TQ), (2 * TQ, 3 * TQ), (3 * TQ, T)):
        for i in range(C_TILES):
            nc.sync.dma_start(
                out=xt_b0[i][:, lo:hi],
                in_=xT[i * 128 : (i + 1) * 128, lo:hi],
            )
    bqk_sb = wpool.tile([128, 4], f32, tag="bqk", name="bqk")
    nc.scalar.dma_start(out=bqk_sb, in_=bqk[:, :])
    # wo is first needed ~100 us in; scalar ring is idle after the w loads.
    wo_tiles = []
    for d in range(2):
        wot = wpool.tile([128, D_MODEL], bf16, tag=f"wo{d}", name=f"wo{d}")
        nc.scalar.dma_start(out=wot, in_=wo[d * 128 : (d + 1) * 128, :])
        wo_tiles.append(wot)
    ones_f = wpool.tile([128, 128], f32, tag="ones_f", name="ones_f")
    nc.vector.memset(ones_f, 1.0)
    ones = wpool.tile([128, 128], bf16, tag="ones", name="ones")
    nc.vector.tensor_copy(ones, ones_f)
    # PE p-state warm-up: ~2.5 us of dummy accumulating matmuls on the ones
    # tile (no DMA dependency) while the first w/x transfers are in flight,
    # so the first real matmuls start on a ramped clock. Sized to end just
    # as the first w/x data lands — overrunning delays real work, and any
    # idle gap after the warm-up resets the p-state ramp.
    ps_warm = ps_o.tile([128, TQ], f32, tag="o", name="warmup")
    for k in range(22):
        nc.tensor.matmul(
            ps_warm[:, 0:128], ones, ones, start=(k == 0), stop=(k == 21)
        )

    def load_x_batch(b, fine_first_chunk):
        """One [128, T] tile per C-row, one dma_start per tile."""
        xts = []
        for i in range(C_TILES):
            xt = xw_pool.tile([128, T], bf16, tag=f"xw{i}", name=f"xw{b}_{i}")
            r0 = i * 128
            c0 = b * T
            col_pieces = [(0, TQ), (TQ, T)] if fine_first_chunk else [(0, T)]
            for lo, hi in col_pieces:
                nc.sync.dma_start(
                    out=xt[:, lo:hi],
                    in_=xT[r0 : r0 + 128, c0 + lo : c0 + hi],
                )
            xts.append(xt)
        return xts

    # persistent per-batch tiles, filled in as the schedule runs
    v_cur = {0: [None] * TK_TILES, 1: [None] * TK_TILES}
    qT = kT = avT = None

    ctx = {
        "nc": nc,
        "f32": f32,
        "bf16": bf16,
        "y": y,
        "y_pool": y_pool,
        "ps_o": ps_o,
        "ps_av": ps_av,
        "wo_tiles": wo_tiles,
    }

    Ident = mybir.ActivationFunctionType.Identity

    def emit_qkt_group(b, ch, xt, js, on_scalar=True):
        """One wide-PSUM group: the two j-columns in `js` (j: 0=qT0, 1=qT1,
        2=kT0, 3=kT1), each 16 matmuls of N=512 + a bias-add. The bias-add
        runs on Scalar during projection phases (Scalar is idle there) and
        on DVE when the group is an attention-phase filler."""
        ps = ps_wide.tile(
            [128, 1024], f32, tag="wide", name=f"psqk{b}_{ch}_{js[0]}{js[1]}"
        )
        for jj, j in enumerate(js):
            psh = ps[:, jj * TQ : (jj + 1) * TQ]
            for i in range(C_TILES):
                nc.tensor.matmul(
                    psh,
                    w_tiles[i][:, j * 128 : (j + 1) * 128],
                    xt[i][:, ch * TQ : (ch + 1) * TQ],
                    start=(i == 0),
                    stop=(i == C_TILES - 1),
                )
            dest = (qT[0], qT[1], kT[0], kT[1])[j]
            if on_scalar:
                nc.scalar.activation(
                    dest[:, ch * TQ : (ch + 1) * TQ],
                    psh,
                    Ident,
                    bias=bqk_sb[:, j : j + 1],
                )
            else:
                nc.vector.tensor_scalar_add(
                    dest[:, ch * TQ : (ch + 1) * TQ], psh, bqk_sb[:, j : j + 1]
                )

    def emit_v_group(b, t_idx, xt, on_scalar=True):
        ps = ps_o.tile([128, TQ], f32, tag="o", name=f"psv{b}_{t_idx}")
        psv = ps[:, :D_LOC]
        for i in range(C_TILES):
            nc.tensor.matmul(
                psv,
                xt[i][:, t_idx * 128 : (t_idx + 1) * 128],
                w_tiles[i][:, 2 * D_LOC : 3 * D_LOC],
                start=(i == 0),
                stop=(i == C_TILES - 1),
            )
        vt = qkv_pool.tile(
            [128, D_LOC], bf16, tag=f"v{t_idx}", name=f"v{t_idx}_{b}", bufs=1
        )
        # ch3 copies go to DVE so Scalar is already drained when the first
        # attention exp arrives.
        if on_scalar:
            nc.scalar.activation(vt, psv, Ident)
        else:
            nc.vector.tensor_copy(vt, psv)
        v_cur[b][t_idx] = vt

    def attn_unit(b, h, ch, avT_u, filler=None, sched=None, prev_fin=None):
        """One (batch, head, chunk) attention unit, software-pipelined:
        scores(p) | AV(p-1) | denominator folds off-PE | filler quanta.
        Returns a finalize closure run by the caller inside the NEXT unit
        (at p==2) so the PE never waits on the fold chains; prev_fin is
        the previous unit's closure, run here."""
        if sched is None:
            sched = [0] * (NPAIR + 1)
        pav = ps_av.tile([128, TQ], f32, tag="av", name=f"pav{b}_{h}_{ch}")
        acc_d = acc_pool.tile([128, TQ], bf16, tag="accd", name=f"accd{b}_{h}_{ch}")
        acc_p = acc_pool.tile([128, TQ], bf16, tag="accp", name=f"accp{b}_{h}_{ch}")
        es_tiles = [None] * NPAIR

        def av_fold(p):
            es = es_tiles[p]
            for jj in range(2):
                tk = 2 * p + jj
                nc.tensor.matmul(
                    pav,
                    v_cur[b][tk][:, h * 128 : (h + 1) * 128],
                    es[:, jj * TQ : (jj + 1) * TQ],
                    start=(tk == 0),
                    stop=(tk == TK_TILES - 1),
                )
            # All folds on DVE: bf16 2x mode runs them at ~0.43 us each, and
            # keeping Pool quiet avoids an SBUF port conflict that degrades
            # concurrent DVE tensor ops ~3x.
            eng, acc = (nc.vector, acc_p) if p < 4 else (nc.vector, acc_d)
            if p in (0, 4):
                eng.tensor_add(acc, es[:, 0:TQ], es[:, TQ : 2 * TQ])
            else:
                eng.tensor_add(acc, acc, es[:, 0:TQ])
                eng.tensor_add(acc, acc, es[:, TQ : 2 * TQ])

        for p in range(NPAIR):
            ps = ps_wide.tile([128, 1024], f32, tag="wide", name=f"pss{b}_{h}_{ch}_{p}")
            for jj in range(2):
                tk = 2 * p + jj
                nc.tensor.matmul(
                    ps[:, jj * TQ : (jj + 1) * TQ],
                    kT[h][:, tk * 128 : (tk + 1) * 128],
                    qT[h][:, ch * TQ : (ch + 1) * TQ],
                    start=True,
                    stop=True,
                )
            es = es_pool.tile([128, 1024], bf16, tag="es", name=f"es{b}_{h}_{ch}_{p}")
            nc.scalar.activation(es, ps, Exp, scale=inv_sqrt_dk)
            es_tiles[p] = es
            # AV lags scores by TWO pairs: exp jitter never reaches the PE
            # (es bufs=4 covers the deeper pipeline).
            if p > 1:
                av_fold(p - 2)
            if p == 2 and prev_fin is not None:
                prev_fin()
            if filler is not None and sched[p]:
                filler(sched[p])
        av_fold(NPAIR - 2)
        if filler is not None and sched[NPAIR]:
            filler(sched[NPAIR])
        av_fold(NPAIR - 1)

        def finalize():
            pdn = ps_o.tile([128, TQ], f32, tag="o", name=f"pdn{b}_{h}_{ch}")
            nc.tensor.matmul(pdn, ones[:, 0:128], acc_d, start=True, stop=False)
            nc.tensor.matmul(pdn, ones[:, 0:128], acc_p, start=False, stop=True)
            rc = rc_pool.tile([128, TQ], f32, tag="rc", name=f"rc{b}_{h}_{ch}")
            nc.vector.reciprocal_approx_fast(out=rc, in_=pdn)
            nc.vector.tensor_mul(avT_u[h][:, ch * TQ : (ch + 1) * TQ], pav, rc)

        return finalize

    # =================== schedule ===================
    class WorkQueue:
        def __init__(self):
            self.q = []

        def push(self, thunks):
            self.q.extend(thunks)

        def emit(self, k):
            for _ in range(k):
                if self.q:
                    self.q.pop(0)()

        def __len__(self):
            return len(self.q)

    def o_quanta(oe):
        return [functools.partial(oe.emit, 1) for _ in range(oe.remaining())]

    wq = WorkQueue()
    pend_fin = None  # previous unit's deferred finalize
    avT_b = {}
    xt_b1 = None

    for b in range(B):
        qT = [
            qkv_pool.tile([128, T], bf16, tag=f"qT{d}", name=f"qT{d}_{b}")
            for d in range(2)
        ]
        kT = [
            qkv_pool.tile([128, T], bf16, tag=f"kT{d}", name=f"kT{d}_{b}")
            for d in range(2)
        ]

        # ---------------- phase P: projections ----------------
        if b == 0:
            xt = xt_b0
            # h0's QKT (j=0,2) + all V; h1's QKT (j=1,3) runs as filler
            # inside the h0 attention units, freeing x earlier.
            # Every chunk is emitted tile-interleaved (j0, j2, v-t0..t3 per
            # x tile) so the PE's per-tile consumption pace (~0.9 us)
            # matches the per-ring DMA delivery pace instead of sprinting
            # ahead during the QKT pass and stalling in the V pass.
            for ch in range(NCH):
                on_scalar = ch < 3
                ps_qk0 = ps_wide.tile(
                    [128, 1024], f32, tag="wide", name=f"psqki_{ch}"
                )
                ps_v = [
                    ps_o.tile([128, TQ], f32, tag="o", name=f"psvi_{ch * 4 + k}")
                    for k in range(2)
                ] + [
                    ps_av.tile([128, TQ], f32, tag="av", name=f"psvi_{ch * 4 + k}")
                    for k in range(2, 4)
                ]
                for i in range(C_TILES):
                    for jj, j in enumerate((0, 2)):
                        nc.tensor.matmul(
                            ps_qk0[:, jj * TQ : (jj + 1) * TQ],
                            w_tiles[i][:, j * 128 : (j + 1) * 128],
                            xt[i][:, ch * TQ : (ch + 1) * TQ],
                            start=(i == 0),
                            stop=(i == C_TILES - 1),
                        )
                    for k in range(4):
                        t = ch * 4 + k
                        nc.tensor.matmul(
                            ps_v[k][:, :D_LOC],
                            xt[i][:, t * 128 : (t + 1) * 128],
                            w_tiles[i][:, 2 * D_LOC : 3 * D_LOC],
                            start=(i == 0),
                            stop=(i == C_TILES - 1),
                        )
                for jj, j in enumerate((0, 2)):
                    dest = (qT[0], qT[1], kT[0], kT[1])[j]
                    # Bias-adds stay on Scalar even for ch3 (they gate the
                    # ps_wide rotation of the first attention unit); only
                    # the V copies move to DVE at the phase tail.
                    nc.scalar.activation(
                        dest[:, ch * TQ : (ch + 1) * TQ],
                        ps_qk0[:, jj * TQ : (jj + 1) * TQ],
                        Ident,
                        bias=bqk_sb[:, j : j + 1],
                    )
                for k in range(4):
                    t = ch * 4 + k
                    vt = qkv_pool.tile(
                        [128, D_LOC], bf16, tag=f"v{t}", name=f"v{t}_0", bufs=1
                    )
                    if on_scalar:
                        nc.scalar.activation(vt, ps_v[k][:, :D_LOC], Ident)
                    else:
                        nc.vector.tensor_copy(vt, ps_v[k][:, :D_LOC])
                    v_cur[0][t] = vt
        else:
            # First QKT(b1) group goes ahead of the deferred finalize so the
            # in-order PE queue is not blocked behind the last b0 unit's
            # DVE fold chain.
            xt = xt_b1
            emit_qkt_group(1, 0, xt, (0, 2))
            pend_fin()  # finalize (b0,1,3): its avT feeds O(b0,c3)
            pend_fin = None
            wq.push(
                o_quanta(
                    OEmitter(
                        ctx, 0, NCH - 1, avT_b[0], cast_engs=("vector", "scalar")
                    )
                )
            )
            for ch in range(NCH):
                if ch > 0:
                    emit_qkt_group(1, ch, xt, (0, 2))
                wq.emit(2)
                emit_qkt_group(1, ch, xt, (1, 3))
                wq.emit(2)
                for ts in range(4):
                    emit_v_group(1, ch * 4 + ts, xt, on_scalar=(ch < 3))
                    if ts < 3:
                        wq.emit(1)

        # ---------------- phase A: attention ----------------
        avT = [
            av_pool.tile([128, T], bf16, tag=f"avT{d}", name=f"avT{d}_{b}")
            for d in range(2)
        ]
        avT_b[b] = avT

        if b == 0:
            # h-outer. Fillers: h1's QKT groups in the h0 units; then
            # O(b0,c) once finalize(1,c) ran.
            wq.push(
                [
                    functools.partial(emit_qkt_group, 0, ch, xt, (1, 3), False)
                    for ch in range(NCH)
                ]
            )
            qsched = [0, 0, 0, 1, 0, 0, 0, 0, 0]  # 1 qkt-h1 group per h0 unit
            osched = {
                (1, 1): [0, 0, 2, 2, 1, 1, 1, 1, 1],  # O(b0,c0) from p2
                (1, 2): [0, 0, 2, 2, 1, 1, 1, 1, 1],
                (1, 3): [0, 0, 2, 2, 1, 1, 1, 1, 1],
            }
            units = [(0, c) for c in range(NCH)] + [(1, c) for c in range(NCH)]
            fins = {}
            for h, c in units:
                if (h, c) == (1, 0):
                    # batch-0 x fully consumed (emission-wise): start
                    # batch-1's x loads into the same tiles.
                    xt_b1 = load_x_batch(1, fine_first_chunk=False)

                def fin_hook(_hc=(h, c)):
                    if pend_fin is not None:
                        pend_fin()
                    ph, pc = fins.get("last", (None, None))
                    if ph == 1:  # finalize of (1,pc) unlocks O(b0,pc)
                        wq.push(
                            o_quanta(
                                OEmitter(
                                    ctx, 0, pc, avT, cast_engs=("vector", "scalar")
                                )
                            )
                        )

                sched = qsched if h == 0 else osched.get((h, c))
                new_fin = attn_unit(b, h, c, avT, wq.emit, sched, prev_fin=fin_hook)
                fins["last"] = (h, c)
                pend_fin = new_fin
        else:
            # ch-outer. All O(b0,*) must be emitted before finalize(b1,0,0)
            # -- it runs at p==1 of unit (1,0); (0,0) drains the leftovers.
            osched = {
                (0, 0): [0, 2, 2, 1, 1, 1, 1, 1, 0],  # b0 leftovers (9)
                (1, 0): [0, 0, 0, 0, 0, 0, 0, 0, 0],
                (0, 1): [0, 0, 2, 2, 1, 1, 1, 1, 0],  # O(b1,c0) from p2
                (1, 1): [0, 1, 1, 1, 1, 1, 1, 1, 1],
                (0, 2): [0, 0, 2, 2, 1, 1, 1, 1, 0],
                (1, 2): [0, 1, 1, 1, 1, 1, 1, 1, 1],
                (0, 3): [0, 0, 2, 2, 1, 1, 1, 1, 0],
                (1, 3): [0, 1, 1, 1, 1, 1, 1, 1, 1],
            }
            units = [(h, c) for c in range(NCH) for h in range(H_LOC)]
            last = [None]
            for h, c in units:

                def fin_hook(_hc=(h, c)):
                    if pend_fin is not None:
                        pend_fin()
                    if last[0] is not None:
                        ph, pc = last[0]
                        if ph == 1:  # O(b1,pc) unlocked
                            wq.push(
                                o_quanta(
                                    OEmitter(
                                        ctx,
                                        1,
                                        pc,
                                        avT,
                                        cast_engs=("vector", "scalar"),
                                    )
                                )
                            )

                new_fin = attn_unit(
                    b, h, c, avT, wq.emit, osched[(h, c)], prev_fin=fin_hook
                )
                last[0] = (h, c)
                pend_fin = new_fin
                if (h, c) == (0, 0):
                    assert len(wq) == 0, len(wq)
            assert len(wq) == 0
            # tail: finalize (1,3) now, then O(b1,c3) with 4-way row-split
            pend_fin()
            pend_fin = None
            ot = OEmitter(
                ctx, 1, NCH - 1, avT, row_split=1, cast_engs=("vector", "scalar")
            )
            ot.store_alt = True
            ot.psum_alt = True
            ot.emit(16)


@functools.cache
def _build():
    from concourse import bacc
    import concourse.tile as tile
    from concourse import mybir

    nc = bacc.Bacc(
        "TRN2",
        target_bir_lowering=False,
        debug=False,
        enable_asserts=False,
        num_devices=N_CORES,
    )
    f32 = mybir.dt.float32
    bf16 = mybir.dt.bfloat16
    xT = nc.dram_tensor("xT", [D_MODEL, BT], bf16, kind="ExternalInput").ap()
    wqkv = nc.dram_tensor(
        "wqkv", [D_MODEL, 3 * D_LOC], bf16, kind="ExternalInput"
    ).ap()
    bqk = nc.dram_tensor("bqk", [128, 4], f32, kind="ExternalInput").ap()
    wo = nc.dram_tensor("wo", [D_LOC, D_MODEL], bf16, kind="ExternalInput").ap()
    y = nc.dram_tensor("y", [BT, D_MODEL], bf16, kind="ExternalOutput").ap()

    with tile.TileContext(nc) as tc:
        with ExitStack() as ctx:
            _body(ctx, tc, xT, wqkv, bqk, wo, y)
    nc.compile()
    return nc


def _shard_inputs(x, Wq, bq, Wk, bk, Wv, bv, Wo, bo):
    """Host-side sharding: returns per-core input maps."""
    import ml_dtypes

    f = np.float32
    b16 = ml_dtypes.bfloat16
    xT = np.ascontiguousarray(np.asarray(x, f).reshape(BT, D_MODEL).T.astype(b16))
    Wq, Wk, Wv, Wo = (np.asarray(a, f) for a in (Wq, Wk, Wv, Wo))
    bq, bk, bv = (np.asarray(a, f) for a in (bq, bk, bv))
    in_maps = []
    for c in range(N_CORES):
        sl = slice(c * D_LOC, (c + 1) * D_LOC)
        wqkv_pad = np.ascontiguousarray(
            np.concatenate([Wq[:, sl], Wk[:, sl], Wv[:, sl]], axis=1).astype(b16)
        )
        bqk_t = np.ascontiguousarray(
            np.stack(
                [
                    bq[sl][:128],
                    bq[sl][128:],
                    bk[sl][:128],
                    bk[sl][128:],
                ],
                axis=1,
            )
        )
        wo_loc = np.ascontiguousarray(Wo[sl, :].astype(b16))
        in_maps.append({"xT": xT, "wqkv": wqkv_pad, "bqk": bqk_t, "wo": wo_loc})
    return in_maps


def _run(in_maps, trace=False, **kwargs):
    from concourse.bass_utils import run_bass_kernel_spmd

    nc = _build()
    return run_bass_kernel_spmd(
        nc, in_maps, core_ids=list(range(N_CORES)), trace=trace, **kwargs
    )


def kernel(x, Wq, bq, Wk, bk, Wv, bv, Wo, bo):
    in_maps = _shard_inputs(x, Wq, bq, Wk, bk, Wv, bv, Wo, bo)
    res = _run(in_maps, trace=False)
    acc = np.zeros((BT, D_MODEL), np.float32)
    for rmap in res.results:
        acc += np.asarray(rmap["y"], dtype=np.float32)
    acc += np.asarray(bo, np.float32)[None, :]
    acc += (np.asarray(bv, np.float32) @ np.asarray(Wo, np.float32))[None, :]
    return acc.reshape(B, T, D_MODEL)

